# revision 1
# baseline (speedup 1.0000x reference)
"""Trainium2 Bass kernel for the CementPINN MLP (dense_mlp, 8 cores).

Data-parallel: x [32768, 8] is sharded along batch into 8 shards of 4096
rows; MLP weights are replicated on every core.  Per core the MLP runs
feature-major (activations h^T [feat, batch]) so every layer is
out^T[m] = sum_k W[k,m]^T @ h^T[k] with the (natural-layout) weight tile as
the stationary operand.  Matmuls run in float32r (full PE rate at N=512).
The physics-constraint clamp is computed batch-major on [128, 32] tiles from
a host-pretransposed copy of x; the raw MLP output [1, 512] per chunk is
bounced through DRAM to convert it to the same batch-major layout.
"""

import numpy as np

import concourse.bacc as bacc
import concourse.mybir as mybir
import concourse.tile as tile
from concourse.bass_utils import run_bass_kernel_spmd

F32 = mybir.dt.float32
F32R = mybir.dt.float32r
AF = mybir.ActivationFunctionType
ALU = mybir.AluOpType

N_CORES = 8
B = 32768
BC = B // N_CORES  # 4096 rows per core
D_IN = 8
H = 1024
P = 128
NB = 512  # batch columns per chunk (= one fp32 PSUM bank)
NCH = BC // NB  # 8 chunks per core
KT = H // P  # 8 feature tiles
JT = BC // P  # 32 batch-major columns

_CACHE = {}


def _build_nc():
    nc = bacc.Bacc("TRN2", target_bir_lowering=False, debug=False)

    xT = nc.declare_dram_parameter("xT", [D_IN, BC], F32R, isOutput=False)
    xc = nc.declare_dram_parameter("xc", [P, D_IN * JT], F32, isOutput=False)
    w1 = nc.declare_dram_parameter("w1", [D_IN, H], F32R, isOutput=False)
    w2 = nc.declare_dram_parameter("w2", [H, H], F32R, isOutput=False)
    w3 = nc.declare_dram_parameter("w3", [H, H], F32R, isOutput=False)
    w4 = nc.declare_dram_parameter("w4", [P, KT], F32R, isOutput=False)
    b1 = nc.declare_dram_parameter("b1", [P, KT], F32, isOutput=False)
    b2 = nc.declare_dram_parameter("b2", [P, KT], F32, isOutput=False)
    b3 = nc.declare_dram_parameter("b3", [P, KT], F32, isOutput=False)
    b4 = nc.declare_dram_parameter("b4", [P, 1], F32, isOutput=False)
    out_d = nc.declare_dram_parameter("out_bm", [P, JT], F32, isOutput=True)

    raw_scratch = nc.dram_tensor("raw_scratch", [NCH, NB], F32)

    with tile.TileContext(nc) as tc:
        with (
            tc.tile_pool(name="wts", bufs=1) as wp,
            tc.tile_pool(name="xin", bufs=1) as xp,
            tc.tile_pool(name="acts", bufs=16) as hp,
            tc.tile_pool(name="raw", bufs=2) as rp,
            tc.tile_pool(name="cst", bufs=1) as cp,
            tc.tile_pool(name="ps", bufs=7, space="PSUM") as pp,
            tc.tile_pool(name="ps4", bufs=1, space="PSUM") as pp4,
        ):
            # ---- w1+b1+xT first on the sync queue: L1 is the only PE
            # work available while the 8MB of W2/W3 streams in, so its
            # inputs must land first.
            w1_sb = wp.tile([P, H], F32R, tag="w1")
            nc.sync.dma_start(w1_sb[:D_IN, :], w1[:])
            b1_sb = wp.tile([P, KT], F32, tag="b1")
            nc.sync.dma_start(b1_sb[:], b1[:])
            xt_sb = xp.tile([P, BC], F32R, tag="xt")
            # chunk 0's columns land as their own small transfer so L1(0)
            # isn't gated on the whole 128KB of x (its completion semaphore
            # arrives several us after the first bytes otherwise).
            nc.sync.dma_start(xt_sb[:D_IN, :NB], xT[:, :NB])
            nc.sync.dma_start(xt_sb[:D_IN, NB:], xT[:, NB:])
            # replicate x / W1 to partition rows 32/64/96 on-chip (cheap
            # SBUF->SBUF DMAs on the idle gpsimd queue) for the row-group
            # packed L1 of chunks >= 2.
            for i in range(1, 4):
                r0 = 32 * i
                nc.gpsimd.dma_start(w1_sb[r0 : r0 + D_IN, :], w1_sb[:D_IN, :])
                nc.gpsimd.dma_start(xt_sb[r0 : r0 + D_IN, :], xt_sb[:D_IN, :])

            # ---- resident weights/biases -------------------------------
            b2_sb = wp.tile([P, KT], F32, tag="b2")
            nc.gpsimd.dma_start(b2_sb[:], b2[:])
            b3_sb = wp.tile([P, KT], F32, tag="b3")
            nc.gpsimd.dma_start(b3_sb[:], b3[:])
            b4_sb = wp.tile([P, 1], F32, tag="b4")
            nc.gpsimd.dma_start(b4_sb[:], b4[:])
            w4_sb = wp.tile([P, KT], F32R, tag="w4")
            nc.gpsimd.dma_start(w4_sb[:], w4[:])
            # w2 then w3 on the sync queue, strictly after w1/b1/xT: the
            # queue is drained in trigger order, so L1's inputs land first
            # and w2 tiles arrive progressively for L2 of chunk 0.
            w2_sb = []
            w3_sb = []
            HH = H // 2
            for k in range(KT):
                t2 = wp.tile([P, H], F32R, tag=f"w2_{k}", name=f"w2sb{k}")
                nc.sync.dma_start(t2[:, :HH], w2[k * P : (k + 1) * P, :HH])
                w2_sb.append(t2)
            for k in range(KT):
                nc.sync.dma_start(w2_sb[k][:, HH:], w2[k * P : (k + 1) * P, HH:])
            for k in range(KT):
                t3 = wp.tile([P, H], F32R, tag=f"w3_{k}", name=f"w3sb{k}")
                nc.sync.dma_start(t3[:], w3[k * P : (k + 1) * P, :])
                w3_sb.append(t3)

            # ---- constraint bounds from x (independent of the MLP) -----
            xc_sb = cp.tile([P, D_IN * JT], F32, tag="xc")
            nc.gpsimd.dma_start(xc_sb[:], xc[:])

            def col(c):
                return xc_sb[:, c * JT : (c + 1) * JT]

            cem, slag, fly, wat, ager = col(0), col(1), col(2), col(3), col(7)

            def ctile(name):
                return cp.tile([P, JT], F32, tag=name, name=name)

            def mtile(name):
                return cp.tile([P, JT], mybir.dt.uint8, tag=name, name=name)

            vec = nc.vector

            age = ctile("age")
            vec.tensor_single_scalar(age[:], ager, 1.0, ALU.max)
            cmask = mtile("cmask")
            vec.tensor_single_scalar(cmask[:], cem, 0.0, ALU.is_gt)
            wmask = mtile("wmask")
            vec.tensor_single_scalar(wmask[:], wat, 0.0, ALU.is_gt)
            vmask = mtile("vmask")
            vec.tensor_tensor(vmask[:], cmask[:], wmask[:], ALU.bitwise_and)
            ones = ctile("ones")
            vec.memset(ones[:], 1.0)
            cems = ctile("cems")
            vec.select(cems[:], cmask[:], cem, ones[:])
            rcem = ctile("rcem")
            vec.reciprocal(rcem[:], cems[:])
            wc = ctile("wc")
            vec.tensor_tensor(wc[:], wat, rcem[:], ALU.mult)
            scm = ctile("scm")
            vec.tensor_tensor(scm[:], slag, fly, ALU.add)
            binder = ctile("binder")
            vec.tensor_tensor(binder[:], cem, scm[:], ALU.add)
            den1 = ctile("den1")
            vec.tensor_single_scalar(den1[:], binder[:], 0.1, ALU.max)
            rden1 = ctile("rden1")
            vec.reciprocal(rden1[:], den1[:])
            r1s = ctile("r1s")
            vec.tensor_tensor(r1s[:], scm[:], rden1[:], ALU.mult)
            amax = ctile("amax")
            vec.tensor_scalar(amax[:], r1s[:], -0.15, 0.95, ALU.mult, ALU.add)
            hyd = ctile("hyd")
            vec.tensor_single_scalar(hyd[:], wc[:], 1.0, ALU.add)
            rhyd = ctile("rhyd")
            vec.reciprocal(rhyd[:], hyd[:])
            ea = ctile("ea")
            vec.tensor_tensor(ea[:], rhyd[:], age[:], ALU.mult)
            ex = ctile("ex")
            nc.scalar.activation(ex[:], ea[:], AF.Exp, scale=-0.01)
            omex = ctile("omex")
            vec.tensor_scalar(omex[:], ex[:], -1.0, 1.0, ALU.mult, ALU.add)
            alpha = ctile("alpha")
            vec.tensor_tensor(alpha[:], amax[:], omex[:], ALU.mult)
            bmask = mtile("bmask")
            vec.tensor_single_scalar(bmask[:], binder[:], 0.0, ALU.is_gt)
            bsafe = ctile("bsafe")
            vec.select(bsafe[:], bmask[:], binder[:], ones[:])
            rbs = ctile("rbs")
            vec.reciprocal(rbs[:], bsafe[:])
            cf = ctile("cf")
            vec.tensor_tensor(cf[:], cem, rbs[:], ALU.mult)
            acf = ctile("acf")
            vec.tensor_tensor(acf[:], alpha[:], cf[:], ALU.mult)
            wcmask = mtile("wcmask")
            vec.tensor_single_scalar(wcmask[:], wc[:], 0.0, ALU.is_gt)
            wcsafe = ctile("wcsafe")
            vec.select(wcsafe[:], wcmask[:], wc[:], ones[:])
            rwcs = ctile("rwcs")
            vec.reciprocal(rwcs[:], wcsafe[:])
            gel = ctile("gel")
            vec.tensor_tensor(gel[:], acf[:], rwcs[:], ALU.mult)
            g = ctile("g")
            vec.tensor_scalar(g[:], gel[:], 0.01, 10.0, ALU.max, ALU.min)
            g2 = ctile("g2")
            vec.tensor_tensor(g2[:], g[:], g[:], ALU.mult)
            g3 = ctile("g3")
            vec.tensor_tensor(g3[:], g2[:], g[:], ALU.mult)
            phys = ctile("phys")
            vec.tensor_scalar(phys[:], g3[:], 50.0, 5.0, ALU.mult, ALU.max)
            physl = ctile("physl")
            vec.tensor_single_scalar(physl[:], phys[:], 120.0, ALU.min)
            tot1 = ctile("tot1")
            vec.tensor_tensor(tot1[:], cem, wat, ALU.add)
            total = ctile("total")
            vec.tensor_tensor(total[:], tot1[:], scm[:], ALU.add)
            dtot = ctile("dtot")
            vec.tensor_single_scalar(dtot[:], total[:], 1e-6, ALU.max)
            rtot = ctile("rtot")
            vec.reciprocal(rtot[:], dtot[:])
            cfac = ctile("cfac")
            vec.tensor_tensor(cfac[:], cem, rtot[:], ALU.mult)
            cons = ctile("cons")
            vec.tensor_single_scalar(cons[:], cfac[:], 120.0, ALU.mult)
            ub = ctile("ub")
            vec.tensor_tensor(ub[:], physl[:], cons[:], ALU.min)
            amask = mtile("amask")
            vec.tensor_tensor(amask[:], vmask[:], bmask[:], ALU.bitwise_and)


            # ---- MLP, feature-major, chunked over batch columns --------
            # L1 is software-pipelined LOOKAHEAD chunks in front: it only
            # needs x + the tiny W1, so the PE chews L1 work while the 4MB
            # W2/W3 streams land, instead of stalling ~13us.
            def emit_l1(c):
                # x and W1 are host-replicated at partitions {0,32,64,96}:
                # chunks >= 2 pack 4 K=8 matmuls into the 4 PE row-groups
                # concurrently (tile_position); chunks 0-1 stay sequential
                # so the PE has steady work while the W2 stream lands.
                h1 = []
                packed = c >= 2
                grp = 4 if packed else 1
                for g in range(KT // grp):
                    pss = []
                    for i in range(grp):
                        m = g * grp + i
                        r0 = 32 * i
                        ps = pp.tile([P, NB], F32, tag="ps", name=f"ps1_{c}_{m}")
                        nc.tensor.matmul(
                            ps[:],
                            w1_sb[r0 : r0 + D_IN, m * P : (m + 1) * P],
                            xt_sb[r0 : r0 + D_IN, c * NB : (c + 1) * NB],
                            start=True,
                            stop=True,
                            tile_position=(r0, 0) if packed else None,
                        )
                        pss.append(ps)
                    for i in range(grp):
                        m = g * grp + i
                        ht = hp.tile([P, NB], F32R, tag="h1", name=f"h1_{c}_{m}", bufs=16)
                        nc.scalar.activation(
                            ht[:], pss[i][:], AF.Relu, bias=b1_sb[:, m : m + 1]
                        )
                        h1.append(ht)
                return h1

            raw_bm = cp.tile([P, JT], F32, tag="raw_bm")
            rawb = ctile("rawb")
            lo5 = ctile("lo5")
            constr = ctile("constr")
            outsb = cp.tile([P, JT], F32, tag="outsb")
            for c in range(NCH):
                h1 = emit_l1(c)
                h2 = []
                for m in range(KT):
                    ps = pp.tile([P, NB], F32, tag="ps", name=f"ps2_{c}_{m}")
                    for k in range(KT):
                        nc.tensor.matmul(
                            ps[:],
                            w2_sb[k][:, m * P : (m + 1) * P],
                            h1[k][:],
                            start=(k == 0),
                            stop=(k == KT - 1),
                        )
                    ht = hp.tile([P, NB], F32R, tag="h2", name=f"h2_{c}_{m}", bufs=12)
                    nc.scalar.activation(ht[:], ps[:], AF.Relu, bias=b2_sb[:, m : m + 1])
                    h2.append(ht)

                h3 = []
                for m in range(KT):
                    ps = pp.tile([P, NB], F32, tag="ps", name=f"ps3_{c}_{m}")
                    for k in range(KT):
                        nc.tensor.matmul(
                            ps[:],
                            w3_sb[k][:, m * P : (m + 1) * P],
                            h2[k][:],
                            start=(k == 0),
                            stop=(k == KT - 1),
                        )
                    ht = hp.tile([P, NB], F32R, tag="h3", name=f"h3_{c}_{m}", bufs=12)
                    nc.scalar.activation(ht[:], ps[:], AF.Relu, bias=b3_sb[:, m : m + 1])
                    h3.append(ht)

                nj = NB // P  # batch-major columns produced by this chunk

                def raw_to_out(ps_part, cols, scr, part_id, eng=None):
                    # psum [1, w] -> DRAM bounce -> batch-major columns of
                    # raw_bm -> clamp -> store, for a slice of this chunk.
                    eng = eng or nc.sync
                    w = cols.stop - cols.start
                    rawt = rp.tile(
                        [1, w], F32, tag="rawt", name=f"rawt{c}_{part_id}"
                    )
                    vec.tensor_copy(rawt[:], ps_part)
                    eng.dma_start(scr, rawt[:])
                    sl = slice(
                        c * nj + cols.start // P, c * nj + cols.stop // P
                    )
                    eng.dma_start(
                        raw_bm[:, sl],
                        scr.rearrange("c (j p) -> p (c j)", p=P),
                    )
                    vec.tensor_single_scalar(
                        rawb[:, sl], raw_bm[:, sl], b4_sb[:, 0:1], ALU.add
                    )
                    vec.tensor_single_scalar(lo5[:, sl], rawb[:, sl], 5.0, ALU.max)
                    vec.tensor_tensor(constr[:, sl], lo5[:, sl], ub[:, sl], ALU.min)
                    vec.select(
                        outsb[:, sl], amask[:, sl], constr[:, sl], rawb[:, sl]
                    )
                    nc.gpsimd.dma_start(out_d[:, sl], outsb[:, sl])

                if c < NCH - 1:
                    ps4 = pp4.tile([1, NB], F32, tag="ps4", name=f"ps4_{c}")
                    for k in range(KT):
                        nc.tensor.matmul(
                            ps4[:],
                            w4_sb[:, k : k + 1],
                            h3[k][:],
                            start=(k == 0),
                            stop=(k == KT - 1),
                        )
                    raw_to_out(
                        ps4[:], slice(0, NB), raw_scratch[c : c + 1, :], "a"
                    )
                else:
                    # last chunk: L4 split into two half-width accumulation
                    # groups so the first half's slow raw conversion overlaps
                    # the second half's matmuls instead of trailing them.
                    HB = NB // 2
                    ps4a = pp4.tile([1, HB], F32, tag="ps4", name="ps4_la")
                    ps4b = pp.tile([1, HB], F32, tag="ps", name="ps4_lb")
                    for k in range(KT):
                        nc.tensor.matmul(
                            ps4a[:],
                            w4_sb[:, k : k + 1],
                            h3[k][:, :HB],
                            start=(k == 0),
                            stop=(k == KT - 1),
                        )
                    raw_to_out(
                        ps4a[:], slice(0, HB), raw_scratch[c : c + 1, :HB], "a"
                    )
                    for k in range(KT):
                        nc.tensor.matmul(
                            ps4b[:],
                            w4_sb[:, k : k + 1],
                            h3[k][:, HB:],
                            start=(k == 0),
                            stop=(k == KT - 1),
                        )
                    raw_to_out(
                        ps4b[:],
                        slice(HB, NB),
                        raw_scratch[c : c + 1, HB:],
                        "b",
                        eng=nc.gpsimd,
                    )

    nc.compile()
    return nc


def _get_nc():
    if "nc" not in _CACHE:
        _CACHE["nc"] = _build_nc()
    return _CACHE["nc"]


def _prep_in_maps(x, W1, b1, W2, b2, W3, b3, W4, b4):
    f = np.float32
    x = np.ascontiguousarray(np.asarray(x, f))
    W1 = np.ascontiguousarray(np.asarray(W1, f))
    W2 = np.ascontiguousarray(np.asarray(W2, f))
    W3 = np.ascontiguousarray(np.asarray(W3, f))
    W4 = np.asarray(W4, f)
    b1p = np.ascontiguousarray(np.asarray(b1, f).reshape(KT, P).T)
    b2p = np.ascontiguousarray(np.asarray(b2, f).reshape(KT, P).T)
    b3p = np.ascontiguousarray(np.asarray(b3, f).reshape(KT, P).T)
    w4p = np.ascontiguousarray(W4.reshape(KT, P).T)
    b4p = np.full((P, 1), np.asarray(b4, f).reshape(-1)[0], f)

    in_maps = []
    for c in range(N_CORES):
        sl = x[c * BC : (c + 1) * BC]  # [4096, 8]
        xT_c = np.ascontiguousarray(sl.T)  # [8, 4096]
        # xc[p, col*JT + j] = sl[j*128 + p, col]
        xc_c = np.ascontiguousarray(
            sl.reshape(JT, P, D_IN).transpose(1, 2, 0).reshape(P, D_IN * JT)
        )
        in_maps.append(
            {
                "xT": xT_c,
                "xc": xc_c,
                "w1": W1,
                "w2": W2,
                "w3": W3,
                "w4": w4p,
                "b1": b1p,
                "b2": b2p,
                "b3": b3p,
                "b4": b4p,
            }
        )
    return in_maps


def kernel(x, W1, b1, W2, b2, W3, b3, W4, b4, **run_kwargs):
    nc = _get_nc()
    in_maps = _prep_in_maps(x, W1, b1, W2, b2, W3, b3, W4, b4)
    res = run_bass_kernel_spmd(nc, in_maps, core_ids=list(range(N_CORES)), **run_kwargs)
    out = np.empty((B, 1), np.float32)
    for c in range(N_CORES):
        out[c * BC : (c + 1) * BC, 0] = res.results[c]["out_bm"].T.reshape(BC)
    if run_kwargs:
        kernel.last_results = res
    return out



# revision 4
# speedup vs baseline: 2.0397x; 2.0397x over previous
"""Trainium2 Bass kernel for the CementPINN MLP (dense_mlp, 8 cores).

Data-parallel: x [32768, 8] is sharded along batch into 8 shards of 4096
rows; MLP weights are replicated on every core.  Per core the MLP runs
feature-major (activations h^T [feat, batch]); every layer is
out^T[m] = sum_k W[k,m]^T @ h^T[k] with the weight tile stationary.

L2/L3/L4 matmuls run in fp8 (e4m3) with MatmulPerfMode.DoubleRow: each
instruction contracts a PAIR of 128-feature k-tiles (stationary [128,2,128],
moving [128,2,512]) at double the fp32r MAC rate.  Host-side the weights are
pre-scaled by powers of two (W2x4, W3x4, W4x16) so the fp8 encoding stays in
the normal range; activations carry the compounded scale (h1'=4h1, h2'=16h2,
h3'=64h3) and each ReLU stage is a single fused instruction:
  ACT:   relu(psum + b')         (bias pre-scaled host-side)
  DVE /Pool: (psum + b') max 0
The raw MLP output is psum/1024 + b4.  ReLU stages round-robin over the
scalar/vector/gpsimd engines so no single engine becomes the bottleneck.
The physics-constraint clamp is computed batch-major on [128, 32] tiles from
a host-pretransposed copy of x; the raw MLP output [1, 512] per chunk is
bounced through DRAM to convert it to the same batch-major layout.
"""

import numpy as np

import concourse.bacc as bacc
import concourse.mybir as mybir
import concourse.tile as tile
from concourse.bass_utils import run_bass_kernel_spmd

F32 = mybir.dt.float32
F32R = mybir.dt.float32r
F8 = mybir.dt.float8e4
AF = mybir.ActivationFunctionType
ALU = mybir.AluOpType
DR = mybir.MatmulPerfMode.DoubleRow

N_CORES = 8
B = 32768
BC = B // N_CORES  # 4096 rows per core
D_IN = 8
H = 1024
P = 128
NB = 512  # batch columns per chunk (= one fp32 PSUM bank)
NCH = BC // NB  # 8 chunks per core
KT = H // P  # 8 feature tiles
QT = KT // 2  # 4 k-tile pairs (DoubleRow)
JT = BC // P  # 32 batch-major columns

# weight pre-scales (powers of two; folded back out via biases / raw stage)
S1 = 4.0  # W1 *= 4 (fp32r, exact)     -> h1' = 4 h1
S2 = 4.0  # W2 *= 4 (fp8)              -> h2' = 16 h2
S3 = 4.0  # W3 *= 4 (fp8)              -> h3' = 64 h3
S4 = 16.0  # W4 *= 16 (fp8)            -> psum4 = 1024 * (W4^T h3)
RAW_DIV = S1 * S2 * S3 * S4  # 1024

_CACHE = {}


def _build_nc():
    nc = bacc.Bacc("TRN2", target_bir_lowering=False, debug=False)

    xT = nc.declare_dram_parameter("xT", [D_IN, BC], F32R, isOutput=False)
    xc = nc.declare_dram_parameter("xc", [P, D_IN * JT], F32, isOutput=False)
    w1 = nc.declare_dram_parameter("w1", [D_IN, H], F32R, isOutput=False)
    w2 = nc.declare_dram_parameter("w2", [P, KT, H], F8, isOutput=False)
    w3 = nc.declare_dram_parameter("w3", [P, KT, H], F8, isOutput=False)
    w4 = nc.declare_dram_parameter("w4", [P, KT, 16], F8, isOutput=False)
    b1 = nc.declare_dram_parameter("b1", [P, KT], F32, isOutput=False)
    b2 = nc.declare_dram_parameter("b2", [P, KT], F32, isOutput=False)
    b3 = nc.declare_dram_parameter("b3", [P, KT], F32, isOutput=False)
    b4 = nc.declare_dram_parameter("b4", [P, 1], F32, isOutput=False)
    out_d = nc.declare_dram_parameter("out_bm", [P, JT], F32, isOutput=True)

    raw_scratch = nc.dram_tensor("raw_scratch", [NCH, NB], F32)

    with tile.TileContext(nc) as tc:
        with (
            tc.tile_pool(name="wts", bufs=1) as wp,
            tc.tile_pool(name="xin", bufs=1) as xp,
            tc.tile_pool(name="acts", bufs=3) as hp,
            tc.tile_pool(name="raw", bufs=2) as rp,
            tc.tile_pool(name="cst", bufs=1) as cp,
            tc.tile_pool(name="ps", bufs=7, space="PSUM") as pp,
            tc.tile_pool(name="ps4", bufs=1, space="PSUM") as pp4,
        ):
            # ---- w1+b1+xT first on the sync queue: L1 is the only PE
            # work available until the W2 pairs land.
            w1_sb = wp.tile([P, H], F32R, tag="w1")
            nc.sync.dma_start(w1_sb[:D_IN, :], w1[:])
            b1_sb = wp.tile([P, KT], F32, tag="b1")
            nc.sync.dma_start(b1_sb[:], b1[:])
            xt_sb = xp.tile([P, BC], F32R, tag="xt")
            # chunk 0's columns land as their own small transfer so L1(0)
            # isn't gated on the whole 128KB of x.
            nc.sync.dma_start(xt_sb[:D_IN, :NB], xT[:, :NB])
            nc.sync.dma_start(xt_sb[:D_IN, NB:], xT[:, NB:])
            # replicate x / W1 to partition rows 32/64/96 on-chip for the
            # row-group packed L1 (tile_position).
            for i in range(1, 4):
                r0 = 32 * i
                nc.gpsimd.dma_start(w1_sb[r0 : r0 + D_IN, :], w1_sb[:D_IN, :])
                nc.gpsimd.dma_start(xt_sb[r0 : r0 + D_IN, :], xt_sb[:D_IN, :])

            # ---- resident weights/biases -------------------------------
            b2_sb = wp.tile([P, KT], F32, tag="b2")
            nc.gpsimd.dma_start(b2_sb[:], b2[:])
            b3_sb = wp.tile([P, KT], F32, tag="b3")
            nc.gpsimd.dma_start(b3_sb[:], b3[:])
            b4_sb = wp.tile([P, 1], F32, tag="b4")
            nc.gpsimd.dma_start(b4_sb[:], b4[:])
            w4_sb = wp.tile([P, KT, 16], F8, tag="w4")
            nc.gpsimd.dma_start(w4_sb[:], w4[:])
            # w2 then w3 on the sync queue, per k-pair so L2(0) can start
            # as soon as its first stationary pair lands.
            w2_sb = wp.tile([P, KT, H], F8, tag="w2")
            for q in range(QT):
                nc.sync.dma_start(
                    w2_sb[:, 2 * q : 2 * q + 2, :], w2[:, 2 * q : 2 * q + 2, :]
                )
            w3_sb = wp.tile([P, KT, H], F8, tag="w3")
            for q in range(QT):
                nc.sync.dma_start(
                    w3_sb[:, 2 * q : 2 * q + 2, :], w3[:, 2 * q : 2 * q + 2, :]
                )

            # ---- constraint bounds from x (independent of the MLP) -----
            xc_sb = cp.tile([P, D_IN * JT], F32, tag="xc")
            nc.gpsimd.dma_start(xc_sb[:], xc[:])

            def col(c):
                return xc_sb[:, c * JT : (c + 1) * JT]

            cem, slag, fly, wat, ager = col(0), col(1), col(2), col(3), col(7)

            def ctile(name):
                return cp.tile([P, JT], F32, tag=name, name=name)

            def mtile(name):
                return cp.tile([P, JT], mybir.dt.uint8, tag=name, name=name)

            vec = nc.vector

            age = ctile("age")
            vec.tensor_single_scalar(age[:], ager, 1.0, ALU.max)
            cmask = mtile("cmask")
            vec.tensor_single_scalar(cmask[:], cem, 0.0, ALU.is_gt)
            wmask = mtile("wmask")
            vec.tensor_single_scalar(wmask[:], wat, 0.0, ALU.is_gt)
            vmask = mtile("vmask")
            vec.tensor_tensor(vmask[:], cmask[:], wmask[:], ALU.bitwise_and)
            ones = ctile("ones")
            vec.memset(ones[:], 1.0)
            cems = ctile("cems")
            vec.select(cems[:], cmask[:], cem, ones[:])
            rcem = ctile("rcem")
            vec.reciprocal(rcem[:], cems[:])
            wc = ctile("wc")
            vec.tensor_tensor(wc[:], wat, rcem[:], ALU.mult)
            scm = ctile("scm")
            vec.tensor_tensor(scm[:], slag, fly, ALU.add)
            binder = ctile("binder")
            vec.tensor_tensor(binder[:], cem, scm[:], ALU.add)
            den1 = ctile("den1")
            vec.tensor_single_scalar(den1[:], binder[:], 0.1, ALU.max)
            rden1 = ctile("rden1")
            vec.reciprocal(rden1[:], den1[:])
            r1s = ctile("r1s")
            vec.tensor_tensor(r1s[:], scm[:], rden1[:], ALU.mult)
            amax = ctile("amax")
            vec.tensor_scalar(amax[:], r1s[:], -0.15, 0.95, ALU.mult, ALU.add)
            hyd = ctile("hyd")
            vec.tensor_single_scalar(hyd[:], wc[:], 1.0, ALU.add)
            rhyd = ctile("rhyd")
            vec.reciprocal(rhyd[:], hyd[:])
            ea = ctile("ea")
            vec.tensor_tensor(ea[:], rhyd[:], age[:], ALU.mult)
            ex = ctile("ex")
            nc.scalar.activation(ex[:], ea[:], AF.Exp, scale=-0.01)
            omex = ctile("omex")
            vec.tensor_scalar(omex[:], ex[:], -1.0, 1.0, ALU.mult, ALU.add)
            alpha = ctile("alpha")
            vec.tensor_tensor(alpha[:], amax[:], omex[:], ALU.mult)
            bmask = mtile("bmask")
            vec.tensor_single_scalar(bmask[:], binder[:], 0.0, ALU.is_gt)
            bsafe = ctile("bsafe")
            vec.select(bsafe[:], bmask[:], binder[:], ones[:])
            rbs = ctile("rbs")
            vec.reciprocal(rbs[:], bsafe[:])
            cf = ctile("cf")
            vec.tensor_tensor(cf[:], cem, rbs[:], ALU.mult)
            acf = ctile("acf")
            vec.tensor_tensor(acf[:], alpha[:], cf[:], ALU.mult)
            wcmask = mtile("wcmask")
            vec.tensor_single_scalar(wcmask[:], wc[:], 0.0, ALU.is_gt)
            wcsafe = ctile("wcsafe")
            vec.select(wcsafe[:], wcmask[:], wc[:], ones[:])
            rwcs = ctile("rwcs")
            vec.reciprocal(rwcs[:], wcsafe[:])
            gel = ctile("gel")
            vec.tensor_tensor(gel[:], acf[:], rwcs[:], ALU.mult)
            g = ctile("g")
            vec.tensor_scalar(g[:], gel[:], 0.01, 10.0, ALU.max, ALU.min)
            g2 = ctile("g2")
            vec.tensor_tensor(g2[:], g[:], g[:], ALU.mult)
            g3 = ctile("g3")
            vec.tensor_tensor(g3[:], g2[:], g[:], ALU.mult)
            phys = ctile("phys")
            vec.tensor_scalar(phys[:], g3[:], 50.0, 5.0, ALU.mult, ALU.max)
            physl = ctile("physl")
            vec.tensor_single_scalar(physl[:], phys[:], 120.0, ALU.min)
            tot1 = ctile("tot1")
            vec.tensor_tensor(tot1[:], cem, wat, ALU.add)
            total = ctile("total")
            vec.tensor_tensor(total[:], tot1[:], scm[:], ALU.add)
            dtot = ctile("dtot")
            vec.tensor_single_scalar(dtot[:], total[:], 1e-6, ALU.max)
            rtot = ctile("rtot")
            vec.reciprocal(rtot[:], dtot[:])
            cfac = ctile("cfac")
            vec.tensor_tensor(cfac[:], cem, rtot[:], ALU.mult)
            cons = ctile("cons")
            vec.tensor_single_scalar(cons[:], cfac[:], 120.0, ALU.mult)
            ub = ctile("ub")
            vec.tensor_tensor(ub[:], physl[:], cons[:], ALU.min)
            amask = mtile("amask")
            vec.tensor_tensor(amask[:], vmask[:], bmask[:], ALU.bitwise_and)

            # ---- ReLU store: split over ACT / DVE (Pool can't read PSUM)
            def relu_store(m, ps, h_t, b_sb):
                dst = h_t[:, m : m + 1, :]
                bcol = b_sb[:, m : m + 1]
                if m % 3 == 1:
                    nc.vector.tensor_scalar(dst, ps[:], bcol, 0.0, ALU.add, ALU.max)
                else:
                    nc.scalar.activation(dst, ps[:], AF.Relu, bias=bcol)

            # ---- MLP, feature-major, chunked over batch columns --------
            def emit_l1(c, h1_t):
                # x and W1 replicated at partitions {0,32,64,96}: pack 4
                # K=8 matmuls into the 4 PE row-groups concurrently.
                for g in range(KT // 4):
                    pss = []
                    for i in range(4):
                        m = g * 4 + i
                        r0 = 32 * i
                        ps = pp.tile([P, NB], F32, tag="ps", name=f"ps1_{c}_{m}")
                        nc.tensor.matmul(
                            ps[:],
                            w1_sb[r0 : r0 + D_IN, m * P : (m + 1) * P],
                            xt_sb[r0 : r0 + D_IN, c * NB : (c + 1) * NB],
                            start=True,
                            stop=True,
                            tile_position=(r0, 0),
                        )
                        pss.append(ps)
                    for i in range(4):
                        relu_store(g * 4 + i, pss[i], h1_t, b1_sb)

            def emit_hidden(c, lname, h_in, h_out, w_sb, b_sb):
                for m in range(KT):
                    ps = pp.tile([P, NB], F32, tag="ps", name=f"ps{lname}_{c}_{m}")
                    for q in range(QT):
                        nc.tensor.matmul(
                            ps[:],
                            w_sb[:, 2 * q : 2 * q + 2, m * P : (m + 1) * P],
                            h_in[:, 2 * q : 2 * q + 2, :],
                            start=(q == 0),
                            stop=(q == QT - 1),
                            perf_mode=DR,
                        )
                    relu_store(m, ps, h_out, b_sb)

            raw_bm = cp.tile([P, JT], F32, tag="raw_bm")
            rawb = ctile("rawb")
            lo5 = ctile("lo5")
            constr = ctile("constr")
            outsb = cp.tile([P, JT], F32, tag="outsb")
            nj = NB // P  # batch-major columns produced per chunk

            def raw_to_out(c, ps_part, cols, scr, part_id, eng=None):
                # psum [1, w] -> DRAM bounce -> batch-major columns of
                # raw_bm -> clamp -> store, for a slice of this chunk.
                eng = eng or nc.sync
                w = cols.stop - cols.start
                rawt = rp.tile([1, w], F32, tag="rawt", name=f"rawt{c}_{part_id}")
                vec.tensor_copy(rawt[:], ps_part)
                eng.dma_start(scr, rawt[:])
                sl = slice(c * nj + cols.start // P, c * nj + cols.stop // P)
                eng.dma_start(
                    raw_bm[:, sl],
                    scr.rearrange("c (j p) -> p (c j)", p=P),
                )
                vec.tensor_scalar(
                    rawb[:, sl], raw_bm[:, sl], 1.0 / RAW_DIV, b4_sb[:, 0:1],
                    ALU.mult, ALU.add,
                )
                vec.tensor_single_scalar(lo5[:, sl], rawb[:, sl], 5.0, ALU.max)
                vec.tensor_tensor(constr[:, sl], lo5[:, sl], ub[:, sl], ALU.min)
                vec.select(outsb[:, sl], amask[:, sl], constr[:, sl], rawb[:, sl])
                nc.gpsimd.dma_start(out_d[:, sl], outsb[:, sl])

            for c in range(NCH):
                h1_t = hp.tile([P, KT, NB], F8, tag="h1", name=f"h1_{c}", bufs=3)
                emit_l1(c, h1_t)
                h2_t = hp.tile([P, KT, NB], F8, tag="h2", name=f"h2_{c}", bufs=3)
                emit_hidden(c, "2", h1_t, h2_t, w2_sb, b2_sb)
                h3_t = hp.tile([P, KT, NB], F8, tag="h3", name=f"h3_{c}", bufs=3)
                emit_hidden(c, "3", h2_t, h3_t, w3_sb, b3_sb)

                if c < NCH - 1:
                    ps4 = pp4.tile([2, NB], F32, tag="ps4", name=f"ps4_{c}")
                    for q in range(QT):
                        nc.tensor.matmul(
                            ps4[:],
                            w4_sb[:, 2 * q : 2 * q + 2, 0:2],
                            h3_t[:, 2 * q : 2 * q + 2, :],
                            start=(q == 0),
                            stop=(q == QT - 1),
                            perf_mode=DR,
                        )
                    raw_to_out(c, ps4[0:1, :], slice(0, NB), raw_scratch[c : c + 1, :], "a")
                else:
                    # last chunk: split L4 into halves so the first half's
                    # slow raw conversion overlaps the second half's matmuls.
                    HB = NB // 2
                    ps4a = pp4.tile([2, HB], F32, tag="ps4", name="ps4_la")
                    ps4b = pp.tile([2, HB], F32, tag="ps", name="ps4_lb")
                    for q in range(QT):
                        nc.tensor.matmul(
                            ps4a[:],
                            w4_sb[:, 2 * q : 2 * q + 2, 0:2],
                            h3_t[:, 2 * q : 2 * q + 2, :HB],
                            start=(q == 0),
                            stop=(q == QT - 1),
                            perf_mode=DR,
                        )
                    raw_to_out(
                        c, ps4a[0:1, :], slice(0, HB), raw_scratch[c : c + 1, :HB], "a"
                    )
                    for q in range(QT):
                        nc.tensor.matmul(
                            ps4b[:],
                            w4_sb[:, 2 * q : 2 * q + 2, 0:2],
                            h3_t[:, 2 * q : 2 * q + 2, HB:],
                            start=(q == 0),
                            stop=(q == QT - 1),
                            perf_mode=DR,
                        )
                    raw_to_out(
                        c,
                        ps4b[0:1, :],
                        slice(HB, NB),
                        raw_scratch[c : c + 1, HB:],
                        "b",
                        eng=nc.gpsimd,
                    )

    nc.compile()
    return nc


def _get_nc():
    if "nc" not in _CACHE:
        _CACHE["nc"] = _build_nc()
    return _CACHE["nc"]


def _prep_in_maps(x, W1, b1, W2, b2, W3, b3, W4, b4):
    f = np.float32
    f8 = mybir.dt.np(F8)
    x = np.ascontiguousarray(np.asarray(x, f))
    W1 = np.asarray(W1, f)
    W2 = np.asarray(W2, f)
    W3 = np.asarray(W3, f)
    W4 = np.asarray(W4, f)
    w1p = np.ascontiguousarray(S1 * W1)
    # [p, k, m] = W[k*128+p, m], scaled + quantized to fp8
    w2p = np.ascontiguousarray(
        (S2 * W2).reshape(KT, P, H).transpose(1, 0, 2).astype(f8)
    )
    w3p = np.ascontiguousarray(
        (S3 * W3).reshape(KT, P, H).transpose(1, 0, 2).astype(f8)
    )
    w4p = np.zeros((P, KT, 16), f8)
    w4c = (S4 * W4).reshape(KT, P).T.astype(f8)
    w4p[:, :, 0] = w4c
    w4p[:, :, 1] = w4c
    b1p = np.ascontiguousarray((S1 * np.asarray(b1, f)).reshape(KT, P).T)
    b2p = np.ascontiguousarray((S1 * S2 * np.asarray(b2, f)).reshape(KT, P).T)
    b3p = np.ascontiguousarray((S1 * S2 * S3 * np.asarray(b3, f)).reshape(KT, P).T)
    b4p = np.full((P, 1), np.asarray(b4, f).reshape(-1)[0], f)

    in_maps = []
    for c in range(N_CORES):
        sl = x[c * BC : (c + 1) * BC]  # [4096, 8]
        xT_c = np.ascontiguousarray(sl.T)  # [8, 4096]
        # xc[p, col*JT + j] = sl[j*128 + p, col]
        xc_c = np.ascontiguousarray(
            sl.reshape(JT, P, D_IN).transpose(1, 2, 0).reshape(P, D_IN * JT)
        )
        in_maps.append(
            {
                "xT": xT_c,
                "xc": xc_c,
                "w1": w1p,
                "w2": w2p,
                "w3": w3p,
                "w4": w4p,
                "b1": b1p,
                "b2": b2p,
                "b3": b3p,
                "b4": b4p,
            }
        )
    return in_maps


def kernel(x, W1, b1, W2, b2, W3, b3, W4, b4, **run_kwargs):
    nc = _get_nc()
    in_maps = _prep_in_maps(x, W1, b1, W2, b2, W3, b3, W4, b4)
    res = run_bass_kernel_spmd(nc, in_maps, core_ids=list(range(N_CORES)), **run_kwargs)
    out = np.empty((B, 1), np.float32)
    for c in range(N_CORES):
        out[c * BC : (c + 1) * BC, 0] = res.results[c]["out_bm"].T.reshape(BC)
    if run_kwargs:
        kernel.last_results = res
    return out


# revision 7
# speedup vs baseline: 2.1594x; 1.0587x over previous
"""Trainium2 Bass kernel for the CementPINN MLP (dense_mlp, 8 cores).

Data-parallel: x [32768, 8] is sharded along batch into 8 shards of 4096
rows; MLP weights are replicated on every core.  Per core the MLP runs
feature-major (activations h^T [feat, batch]); every layer is
out^T[m] = sum_k W[k,m]^T @ h^T[k] with the weight tile stationary.

L2/L3/L4 matmuls run in fp8 (e4m3) with MatmulPerfMode.DoubleRow: each
instruction contracts a PAIR of 128-feature k-tiles (stationary [128,2,128],
moving [128,2,512]) at ~1.5x the fp32r MAC rate.  Host-side the weights are
pre-scaled by powers of two (W2x4, W3x4, W4x16) so the fp8 encoding stays in
the normal range; activations carry the compounded scale (h1'=4h1, h2'=16h2,
h3'=64h3) and each ReLU stage is one fused instruction:
  ACT:  relu(psum + b')        DVE: (psum + b') max 0   (biases pre-scaled)
The raw MLP output is psum/1024 + b4.  ReLU stages split evenly over the
scalar/vector engines.  L1 (K=8, fp32r) is packed 4-wide into PE row groups
via tile_position and software-pipelined two chunks ahead so the PE always
has independent work across layer boundaries.  The physics-constraint clamp
is computed batch-major on [128, 32] tiles from a host-pretransposed copy of
x; the per-chunk raw row [1, 512] is converted to batch-major with a single
SBUF->SBUF strided DMA (no DRAM bounce).
"""

import numpy as np

import concourse.bacc as bacc
import concourse.mybir as mybir
import concourse.tile as tile
from concourse.bass_utils import run_bass_kernel_spmd

F32 = mybir.dt.float32
F32R = mybir.dt.float32r
F8 = mybir.dt.float8e4
AF = mybir.ActivationFunctionType
ALU = mybir.AluOpType
DR = mybir.MatmulPerfMode.DoubleRow

N_CORES = 8
B = 32768
BC = B // N_CORES  # 4096 rows per core
D_IN = 8
H = 1024
P = 128
NB = 512  # batch columns per chunk (= one fp32 PSUM bank)
NCH = BC // NB  # 8 chunks per core
KT = H // P  # 8 feature tiles
QT = KT // 2  # 4 k-tile pairs (DoubleRow)
JT = BC // P  # 32 batch-major columns

# weight pre-scales (powers of two; folded back out via biases / raw stage)
S1 = 4.0  # W1 *= 4 (fp32r, exact)     -> h1' = 4 h1
S2 = 4.0  # W2 *= 4 (fp8)              -> h2' = 16 h2
S3 = 4.0  # W3 *= 4 (fp8)              -> h3' = 64 h3
S4 = 16.0  # W4 *= 16 (fp8)            -> psum4 = 1024 * (W4^T h3)
RAW_DIV = S1 * S2 * S3 * S4  # 1024

_CACHE = {}


def _build_nc():
    nc = bacc.Bacc("TRN2", target_bir_lowering=False, debug=False)

    xT = nc.declare_dram_parameter("xT", [D_IN, BC], F32R, isOutput=False)
    xc = nc.declare_dram_parameter("xc", [P, D_IN * JT], F32, isOutput=False)
    w1 = nc.declare_dram_parameter("w1", [D_IN, H], F32R, isOutput=False)
    w2 = nc.declare_dram_parameter("w2", [P, KT, H], F8, isOutput=False)
    w3 = nc.declare_dram_parameter("w3", [P, KT, H], F8, isOutput=False)
    w4 = nc.declare_dram_parameter("w4", [P, KT, 16], F8, isOutput=False)
    b1 = nc.declare_dram_parameter("b1", [P, KT], F32, isOutput=False)
    b23 = nc.declare_dram_parameter("b23", [P, 2 * KT], F32, isOutput=False)
    b4 = nc.declare_dram_parameter("b4", [P, 1], F32, isOutput=False)
    out_d = nc.declare_dram_parameter("out_bm", [P, JT], F32, isOutput=True)

    with tile.TileContext(nc) as tc:
        with (
            tc.tile_pool(name="wts", bufs=1) as wp,
            tc.tile_pool(name="xin", bufs=1) as xp,
            tc.tile_pool(name="acts", bufs=3) as hp,
            tc.tile_pool(name="raw", bufs=2) as rp,
            tc.tile_pool(name="cst", bufs=1) as cp,
            tc.tile_pool(name="ps", bufs=7, space="PSUM") as pp,
            tc.tile_pool(name="ps4", bufs=1, space="PSUM") as pp4,
        ):
            # ---- sync DMA queue: L1's inputs first, then W2 pair 0 so
            # L2(0) can start while the rest streams.
            w1_sb = wp.tile([P, H], F32R, tag="w1")
            nc.sync.dma_start(w1_sb[:D_IN, :], w1[:])
            xt_sb = xp.tile([P, BC], F32R, tag="xt")
            nc.sync.dma_start(xt_sb[:D_IN, :NB], xT[:, :NB])
            b1_sb = wp.tile([P, KT], F32, tag="b1")
            nc.sync.dma_start(b1_sb[:], b1[:])
            w2_sb = wp.tile([P, KT, H], F8, tag="w2")
            nc.sync.dma_start(w2_sb[:, 0:2, :], w2[:, 0:2, :])
            nc.sync.dma_start(xt_sb[:D_IN, NB:], xT[:, NB:])
            b23_sb = wp.tile([P, 2 * KT], F32, tag="b23")
            nc.sync.dma_start(b23_sb[:], b23[:])
            for q in range(1, QT):
                nc.sync.dma_start(
                    w2_sb[:, 2 * q : 2 * q + 2, :], w2[:, 2 * q : 2 * q + 2, :]
                )
            w3_sb = wp.tile([P, KT, H], F8, tag="w3")
            for q in range(QT):
                nc.sync.dma_start(
                    w3_sb[:, 2 * q : 2 * q + 2, :], w3[:, 2 * q : 2 * q + 2, :]
                )

            # ---- gpsimd DMA queue: xc (constraint inputs) first, then the
            # on-chip x/W1 replicas for the packed L1 of chunks >= 1.
            xc_sb = cp.tile([P, D_IN * JT], F32, tag="xc")
            nc.gpsimd.dma_start(xc_sb[:], xc[:])
            for i in range(1, 4):
                r0 = 32 * i
                nc.gpsimd.dma_start(w1_sb[r0 : r0 + D_IN, :], w1_sb[:D_IN, :])
            for i in range(1, 4):
                r0 = 32 * i
                nc.gpsimd.dma_start(xt_sb[r0 : r0 + D_IN, NB:], xt_sb[:D_IN, NB:])
            b4_sb = wp.tile([P, 1], F32, tag="b4")
            nc.gpsimd.dma_start(b4_sb[:], b4[:])
            w4_sb = wp.tile([P, KT, 16], F8, tag="w4")
            nc.gpsimd.dma_start(w4_sb[:], w4[:])

            # ---- ReLU store: even m on ACT, odd m on DVE ---------------
            def relu_store(m, ps, h_t, b_sb, boff):
                dst = h_t[:, m : m + 1, :]
                bcol = b_sb[:, boff + m : boff + m + 1]
                if m % 2 == 1:
                    nc.vector.tensor_scalar(dst, ps[:], bcol, 0.0, ALU.add, ALU.max)
                else:
                    nc.scalar.activation(dst, ps[:], AF.Relu, bias=bcol)

            h_tiles = {}

            def emit_l1(c):
                # chunk 0 runs unpacked on PE rows 0-7 (depends only on w1 +
                # x chunk 0); later chunks pack 4 K=8 matmuls into the PE
                # row groups via the on-chip replicas.
                h1_t = hp.tile([P, KT, NB], F8, tag="h1", name=f"h1_{c}", bufs=3)
                h_tiles[("h1", c)] = h1_t
                packed = c >= 1
                grp = 4 if packed else 1
                for g in range(KT // grp):
                    pss = []
                    for i in range(grp):
                        m = g * grp + i
                        r0 = 32 * i
                        ps = pp.tile([P, NB], F32, tag="ps", name=f"ps1_{c}_{m}")
                        nc.tensor.matmul(
                            ps[:],
                            w1_sb[r0 : r0 + D_IN, m * P : (m + 1) * P],
                            xt_sb[r0 : r0 + D_IN, c * NB : (c + 1) * NB],
                            start=True,
                            stop=True,
                            tile_position=(r0, 0) if packed else None,
                        )
                        pss.append(ps)
                    for i in range(grp):
                        relu_store(g * grp + i, pss[i], h1_t, b1_sb, 0)

            def emit_hidden(c, lname, h_in, h_out, w_sb, b_sb, boff):
                for m in range(KT):
                    ps = pp.tile([P, NB], F32, tag="ps", name=f"ps{lname}_{c}_{m}")
                    for q in range(QT):
                        nc.tensor.matmul(
                            ps[:],
                            w_sb[:, 2 * q : 2 * q + 2, m * P : (m + 1) * P],
                            h_in[:, 2 * q : 2 * q + 2, :],
                            start=(q == 0),
                            stop=(q == QT - 1),
                            perf_mode=DR,
                        )
                    relu_store(m, ps, h_out, b_sb, boff)

            # ---- L1 software-pipelined two chunks ahead ----------------
            emit_l1(0)
            emit_l1(1)

            # ---- constraint bounds from x (independent of the MLP).
            # Emitted here so the DVE-queue work lands after chunk 0/1's h1
            # ReLUs but well before the first raw conversion needs `ub`.
            def col(c):
                return xc_sb[:, c * JT : (c + 1) * JT]

            cem, slag, fly, wat, ager = col(0), col(1), col(2), col(3), col(7)

            def ctile(name):
                return cp.tile([P, JT], F32, tag=name, name=name)

            def mtile(name):
                return cp.tile([P, JT], mybir.dt.uint8, tag=name, name=name)

            vec = nc.vector

            age = ctile("age")
            vec.tensor_single_scalar(age[:], ager, 1.0, ALU.max)
            cmask = mtile("cmask")
            vec.tensor_single_scalar(cmask[:], cem, 0.0, ALU.is_gt)
            wmask = mtile("wmask")
            vec.tensor_single_scalar(wmask[:], wat, 0.0, ALU.is_gt)
            vmask = mtile("vmask")
            vec.tensor_tensor(vmask[:], cmask[:], wmask[:], ALU.bitwise_and)
            ones = ctile("ones")
            vec.memset(ones[:], 1.0)
            cems = ctile("cems")
            vec.select(cems[:], cmask[:], cem, ones[:])
            rcem = ctile("rcem")
            vec.reciprocal(rcem[:], cems[:])
            wc = ctile("wc")
            vec.tensor_tensor(wc[:], wat, rcem[:], ALU.mult)
            scm = ctile("scm")
            vec.tensor_tensor(scm[:], slag, fly, ALU.add)
            binder = ctile("binder")
            vec.tensor_tensor(binder[:], cem, scm[:], ALU.add)
            den1 = ctile("den1")
            vec.tensor_single_scalar(den1[:], binder[:], 0.1, ALU.max)
            rden1 = ctile("rden1")
            vec.reciprocal(rden1[:], den1[:])
            r1s = ctile("r1s")
            vec.tensor_tensor(r1s[:], scm[:], rden1[:], ALU.mult)
            amax = ctile("amax")
            vec.tensor_scalar(amax[:], r1s[:], -0.15, 0.95, ALU.mult, ALU.add)
            hyd = ctile("hyd")
            vec.tensor_single_scalar(hyd[:], wc[:], 1.0, ALU.add)
            rhyd = ctile("rhyd")
            vec.reciprocal(rhyd[:], hyd[:])
            ea = ctile("ea")
            vec.tensor_tensor(ea[:], rhyd[:], age[:], ALU.mult)
            ex = ctile("ex")
            nc.scalar.activation(ex[:], ea[:], AF.Exp, scale=-0.01)
            omex = ctile("omex")
            vec.tensor_scalar(omex[:], ex[:], -1.0, 1.0, ALU.mult, ALU.add)
            alpha = ctile("alpha")
            vec.tensor_tensor(alpha[:], amax[:], omex[:], ALU.mult)
            bmask = mtile("bmask")
            vec.tensor_single_scalar(bmask[:], binder[:], 0.0, ALU.is_gt)
            bsafe = ctile("bsafe")
            vec.select(bsafe[:], bmask[:], binder[:], ones[:])
            rbs = ctile("rbs")
            vec.reciprocal(rbs[:], bsafe[:])
            cf = ctile("cf")
            vec.tensor_tensor(cf[:], cem, rbs[:], ALU.mult)
            acf = ctile("acf")
            vec.tensor_tensor(acf[:], alpha[:], cf[:], ALU.mult)
            wcmask = mtile("wcmask")
            vec.tensor_single_scalar(wcmask[:], wc[:], 0.0, ALU.is_gt)
            wcsafe = ctile("wcsafe")
            vec.select(wcsafe[:], wcmask[:], wc[:], ones[:])
            rwcs = ctile("rwcs")
            vec.reciprocal(rwcs[:], wcsafe[:])
            gel = ctile("gel")
            vec.tensor_tensor(gel[:], acf[:], rwcs[:], ALU.mult)
            g = ctile("g")
            vec.tensor_scalar(g[:], gel[:], 0.01, 10.0, ALU.max, ALU.min)
            g2 = ctile("g2")
            vec.tensor_tensor(g2[:], g[:], g[:], ALU.mult)
            g3 = ctile("g3")
            vec.tensor_tensor(g3[:], g2[:], g[:], ALU.mult)
            phys = ctile("phys")
            vec.tensor_scalar(phys[:], g3[:], 50.0, 5.0, ALU.mult, ALU.max)
            physl = ctile("physl")
            vec.tensor_single_scalar(physl[:], phys[:], 120.0, ALU.min)
            tot1 = ctile("tot1")
            vec.tensor_tensor(tot1[:], cem, wat, ALU.add)
            total = ctile("total")
            vec.tensor_tensor(total[:], tot1[:], scm[:], ALU.add)
            dtot = ctile("dtot")
            vec.tensor_single_scalar(dtot[:], total[:], 1e-6, ALU.max)
            rtot = ctile("rtot")
            vec.reciprocal(rtot[:], dtot[:])
            cfac = ctile("cfac")
            vec.tensor_tensor(cfac[:], cem, rtot[:], ALU.mult)
            cons = ctile("cons")
            vec.tensor_single_scalar(cons[:], cfac[:], 120.0, ALU.mult)
            ub = ctile("ub")
            vec.tensor_tensor(ub[:], physl[:], cons[:], ALU.min)
            amask = mtile("amask")
            vec.tensor_tensor(amask[:], vmask[:], bmask[:], ALU.bitwise_and)

            # ---- raw [1, w] -> batch-major clamp -> store --------------
            raw_bm = cp.tile([P, JT], F32, tag="raw_bm")
            rawb = ctile("rawb")
            lo5 = ctile("lo5")
            constr = ctile("constr")
            outsb = cp.tile([P, JT], F32, tag="outsb")
            nj = NB // P  # batch-major columns produced per chunk

            def raw_to_out(c, ps_part, rows, part_id, eng=None):
                # batch-major mapping: sample n of chunk c lives at
                # (partition n//4, column c*4 + n%4).  `rows` selects the
                # partition range this piece covers (full or half chunk).
                eng = eng or nc.sync
                w = (rows.stop - rows.start) * nj
                rawt = rp.tile([1, w], F32, tag="rawt", name=f"rawt{c}_{part_id}")
                vec.tensor_copy(rawt[:], ps_part)
                sl = slice(c * nj, (c + 1) * nj)
                # single strided SBUF->SBUF DMA does the [1, w] -> [p, 4]
                # batch-major transpose in one hop.
                eng.dma_start(
                    raw_bm[rows, sl],
                    rawt[0:1, :].rearrange("o (p j) -> o p j", j=nj),
                )
                vec.tensor_scalar(
                    rawb[rows, sl], raw_bm[rows, sl], 1.0 / RAW_DIV,
                    b4_sb[rows, 0:1], ALU.mult, ALU.add,
                )
                vec.tensor_single_scalar(lo5[rows, sl], rawb[rows, sl], 5.0, ALU.max)
                vec.tensor_tensor(
                    constr[rows, sl], lo5[rows, sl], ub[rows, sl], ALU.min
                )
                vec.select(
                    outsb[rows, sl], amask[rows, sl], constr[rows, sl],
                    rawb[rows, sl],
                )
                nc.gpsimd.dma_start(out_d[rows, sl], outsb[rows, sl])

            # ---- main chunk loop ---------------------------------------
            for c in range(NCH):
                h1_t = h_tiles[("h1", c)]
                h2_t = hp.tile([P, KT, NB], F8, tag="h2", name=f"h2_{c}", bufs=3)
                emit_hidden(c, "2", h1_t, h2_t, w2_sb, b23_sb, 0)
                if c + 2 < NCH:
                    # L1 lookahead fills the PE while L2's last ReLUs land.
                    emit_l1(c + 2)
                h3_t = hp.tile([P, KT, NB], F8, tag="h3", name=f"h3_{c}", bufs=3)
                emit_hidden(c, "3", h2_t, h3_t, w3_sb, b23_sb, KT)

                if c < NCH - 1:
                    ps4 = pp4.tile([2, NB], F32, tag="ps4", name=f"ps4_{c}")
                    for q in range(QT):
                        nc.tensor.matmul(
                            ps4[:],
                            w4_sb[:, 2 * q : 2 * q + 2, 0:2],
                            h3_t[:, 2 * q : 2 * q + 2, :],
                            start=(q == 0),
                            stop=(q == QT - 1),
                            perf_mode=DR,
                        )
                    raw_to_out(c, ps4[0:1, :], slice(0, P), "a")
                else:
                    # last chunk: split L4 into halves so the first half's
                    # raw conversion overlaps the second half's matmuls.
                    HB = NB // 2
                    ps4a = pp4.tile([2, HB], F32, tag="ps4", name="ps4_la")
                    ps4b = pp.tile([2, HB], F32, tag="ps", name="ps4_lb")
                    for q in range(QT):
                        nc.tensor.matmul(
                            ps4a[:],
                            w4_sb[:, 2 * q : 2 * q + 2, 0:2],
                            h3_t[:, 2 * q : 2 * q + 2, :HB],
                            start=(q == 0),
                            stop=(q == QT - 1),
                            perf_mode=DR,
                        )
                    raw_to_out(c, ps4a[0:1, :], slice(0, P // 2), "a")
                    for q in range(QT):
                        nc.tensor.matmul(
                            ps4b[:],
                            w4_sb[:, 2 * q : 2 * q + 2, 0:2],
                            h3_t[:, 2 * q : 2 * q + 2, HB:],
                            start=(q == 0),
                            stop=(q == QT - 1),
                            perf_mode=DR,
                        )
                    raw_to_out(c, ps4b[0:1, :], slice(P // 2, P), "b", eng=nc.gpsimd)

    nc.compile()
    return nc


def _get_nc():
    if "nc" not in _CACHE:
        _CACHE["nc"] = _build_nc()
    return _CACHE["nc"]


def _prep_in_maps(x, W1, b1, W2, b2, W3, b3, W4, b4):
    f = np.float32
    f8 = mybir.dt.np(F8)
    x = np.ascontiguousarray(np.asarray(x, f))
    W1 = np.asarray(W1, f)
    W2 = np.asarray(W2, f)
    W3 = np.asarray(W3, f)
    W4 = np.asarray(W4, f)
    w1p = np.ascontiguousarray(S1 * W1)
    # [p, k, m] = W[k*128+p, m], scaled + quantized to fp8
    w2p = np.ascontiguousarray(
        (S2 * W2).reshape(KT, P, H).transpose(1, 0, 2).astype(f8)
    )
    w3p = np.ascontiguousarray(
        (S3 * W3).reshape(KT, P, H).transpose(1, 0, 2).astype(f8)
    )
    w4p = np.zeros((P, KT, 16), f8)
    w4c = (S4 * W4).reshape(KT, P).T.astype(f8)
    w4p[:, :, 0] = w4c
    w4p[:, :, 1] = w4c
    b1p = np.ascontiguousarray((S1 * np.asarray(b1, f)).reshape(KT, P).T)
    b23p = np.concatenate(
        [
            (S1 * S2 * np.asarray(b2, f)).reshape(KT, P).T,
            (S1 * S2 * S3 * np.asarray(b3, f)).reshape(KT, P).T,
        ],
        axis=1,
    )
    b23p = np.ascontiguousarray(b23p)
    b4p = np.full((P, 1), np.asarray(b4, f).reshape(-1)[0], f)

    in_maps = []
    for c in range(N_CORES):
        sl = x[c * BC : (c + 1) * BC]  # [4096, 8]
        xT_c = np.ascontiguousarray(sl.T)  # [8, 4096]
        # xc[p, f*JT + c*4 + j] = sl[c*512 + p*4 + j, f]
        xc_c = np.ascontiguousarray(
            sl.reshape(NCH, P, NB // P, D_IN)
            .transpose(1, 3, 0, 2)
            .reshape(P, D_IN * JT)
        )
        in_maps.append(
            {
                "xT": xT_c,
                "xc": xc_c,
                "w1": w1p,
                "w2": w2p,
                "w3": w3p,
                "w4": w4p,
                "b1": b1p,
                "b23": b23p,
                "b4": b4p,
            }
        )
    return in_maps


def kernel(x, W1, b1, W2, b2, W3, b3, W4, b4, **run_kwargs):
    nc = _get_nc()
    in_maps = _prep_in_maps(x, W1, b1, W2, b2, W3, b3, W4, b4)
    res = run_bass_kernel_spmd(nc, in_maps, core_ids=list(range(N_CORES)), **run_kwargs)
    out = np.empty((B, 1), np.float32)
    for c in range(N_CORES):
        out[c * BC : (c + 1) * BC, 0] = (
            res.results[c]["out_bm"]
            .reshape(P, NCH, NB // P)
            .transpose(1, 0, 2)
            .reshape(BC)
        )
    if run_kwargs:
        kernel.last_results = res
    return out


# revision 9
# speedup vs baseline: 2.2154x; 1.0259x over previous
"""Trainium2 Bass kernel for the CementPINN MLP (dense_mlp, 8 cores).

Data-parallel: x [32768, 8] is sharded along batch into 8 shards of 4096
rows; MLP weights are replicated on every core.  Per core the MLP runs
feature-major (activations h^T [feat, batch]); every layer is
out^T[m] = sum_k W[k,m]^T @ h^T[k] with the weight tile stationary.

L2/L3/L4 matmuls run in fp8 (e4m3) with MatmulPerfMode.DoubleRow: each
instruction contracts a PAIR of 128-feature k-tiles (stationary [128,2,128],
moving [128,2,512]) at ~1.5x the fp32r MAC rate.  Host-side the weights are
pre-scaled by powers of two (W2x4, W3x4, W4x16) so the fp8 encoding stays in
the normal range; activations carry the compounded scale (h1'=4h1, h2'=16h2,
h3'=64h3) and each ReLU stage is one fused instruction:
  ACT:  relu(psum + b')        DVE: (psum + b') max 0   (biases pre-scaled)
The raw MLP output is psum/1024 + b4.  ReLU stages split evenly over the
scalar/vector engines.  L1 (K=8, fp32r) is packed 4-wide into PE row groups
via tile_position and software-pipelined two chunks ahead so the PE always
has independent work across layer boundaries.  The physics-constraint clamp
is computed batch-major on [128, 32] tiles from a host-pretransposed copy of
x; the per-chunk raw row [1, 512] is converted to batch-major with a single
SBUF->SBUF strided DMA (no DRAM bounce).
"""

import numpy as np

import concourse.bacc as bacc
import concourse.mybir as mybir
import concourse.tile as tile
from concourse.bass_utils import run_bass_kernel_spmd

F32 = mybir.dt.float32
F32R = mybir.dt.float32r
F8 = mybir.dt.float8e4
AF = mybir.ActivationFunctionType
ALU = mybir.AluOpType
DR = mybir.MatmulPerfMode.DoubleRow

N_CORES = 8
B = 32768
BC = B // N_CORES  # 4096 rows per core
D_IN = 8
H = 1024
P = 128
NB = 512  # batch columns per chunk (= one fp32 PSUM bank)
NCH = BC // NB  # 8 chunks per core
KT = H // P  # 8 feature tiles
QT = KT // 2  # 4 k-tile pairs (DoubleRow)
JT = BC // P  # 32 batch-major columns

# weight pre-scales (powers of two; folded back out via biases / raw stage)
S1 = 4.0  # W1 *= 4 (fp32r, exact)     -> h1' = 4 h1
S2 = 4.0  # W2 *= 4 (fp8)              -> h2' = 16 h2
S3 = 4.0  # W3 *= 4 (fp8)              -> h3' = 64 h3
S4 = 16.0  # W4 *= 16 (fp8)            -> psum4 = 1024 * (W4^T h3)
RAW_DIV = S1 * S2 * S3 * S4  # 1024

_CACHE = {}


def _build_nc():
    nc = bacc.Bacc("TRN2", target_bir_lowering=False, debug=False)

    NCONST = D_IN * JT + KT + 2 * KT + 1  # xc | b1 | b23 | b4
    wx = nc.declare_dram_parameter("wx", [P, H + BC], F32R, isOutput=False)
    consts = nc.declare_dram_parameter("consts", [P, NCONST], F32, isOutput=False)
    w2 = nc.declare_dram_parameter("w2", [P, KT, H], F8, isOutput=False)
    w3 = nc.declare_dram_parameter("w3", [P, KT, H], F8, isOutput=False)
    w4 = nc.declare_dram_parameter("w4", [P, KT, 16], F8, isOutput=False)
    out_d = nc.declare_dram_parameter("out_bm", [P, JT], F32, isOutput=True)

    with tile.TileContext(nc) as tc:
        with (
            tc.tile_pool(name="wts", bufs=1) as wp,
            tc.tile_pool(name="xin", bufs=1) as xp,
            tc.tile_pool(name="acts", bufs=3) as hp,
            tc.tile_pool(name="raw", bufs=2) as rp,
            tc.tile_pool(name="cst", bufs=1) as cp,
            tc.tile_pool(name="ps", bufs=7, space="PSUM") as pp,
            tc.tile_pool(name="ps4", bufs=1, space="PSUM") as pp4,
        ):
            # ---- sync (HW DGE) queue: x/W1 (host-replicated into the 4 PE
            # row groups) + the packed constants.  Chunk-0 columns land
            # first so L1(0) starts early; chunks 1-2 next for the L1
            # lookahead; the rest follows.
            wx_sb = wp.tile([P, H + BC], F32R, tag="wx")
            nc.sync.dma_start(wx_sb[:, : H + NB], wx[:, : H + NB])
            cs_sb = cp.tile([P, NCONST], F32, tag="consts")
            nc.sync.dma_start(cs_sb[:], consts[:])
            nc.sync.dma_start(
                wx_sb[:, H + NB : H + 3 * NB], wx[:, H + NB : H + 3 * NB]
            )
            nc.sync.dma_start(wx_sb[:, H + 3 * NB :], wx[:, H + 3 * NB :])
            w1_sb = wx_sb[:, :H]
            xc_sb = cs_sb[:, : D_IN * JT]
            b1_sb = cs_sb[:, D_IN * JT : D_IN * JT + KT]
            b23_sb = cs_sb[:, D_IN * JT + KT : D_IN * JT + 3 * KT]
            b4_sb = cs_sb[:, D_IN * JT + 3 * KT :]

            # ---- scalar (HW DGE) queue: weights stream in parallel -----
            w2_sb = wp.tile([P, KT, H], F8, tag="w2")
            nc.scalar.dma_start(w2_sb[:, 0:2, :], w2[:, 0:2, :])
            nc.scalar.dma_start(w2_sb[:, 2:, :], w2[:, 2:, :])
            w3_sb = wp.tile([P, KT, H], F8, tag="w3")
            nc.scalar.dma_start(w3_sb[:], w3[:])

            # ---- gpsimd queue: only the tiny L4 weight ----------------
            w4_sb = wp.tile([P, KT, 16], F8, tag="w4")
            nc.gpsimd.dma_start(w4_sb[:], w4[:])

            # ---- ReLU store: even m on ACT, odd m on DVE ---------------
            def relu_store(m, ps, h_t, b_sb, boff):
                dst = h_t[:, m : m + 1, :]
                bcol = b_sb[:, boff + m : boff + m + 1]
                if m % 2 == 1:
                    nc.vector.tensor_scalar(dst, ps[:], bcol, 0.0, ALU.add, ALU.max)
                else:
                    nc.scalar.activation(dst, ps[:], AF.Relu, bias=bcol)

            h_tiles = {}

            def emit_l1(c):
                # 4 K=8 matmuls packed into the PE row groups (x/W1 are
                # host-replicated at partitions 0/32/64/96).
                h1_t = hp.tile([P, KT, NB], F8, tag="h1", name=f"h1_{c}", bufs=3)
                h_tiles[("h1", c)] = h1_t
                packed = True
                grp = 4
                for g in range(KT // grp):
                    pss = []
                    for i in range(grp):
                        m = g * grp + i
                        r0 = 32 * i
                        ps = pp.tile([P, NB], F32, tag="ps", name=f"ps1_{c}_{m}")
                        nc.tensor.matmul(
                            ps[:],
                            w1_sb[r0 : r0 + D_IN, m * P : (m + 1) * P],
                            wx_sb[r0 : r0 + D_IN, H + c * NB : H + (c + 1) * NB],
                            start=True,
                            stop=True,
                            tile_position=(r0, 0) if packed else None,
                        )
                        pss.append(ps)
                    for i in range(grp):
                        relu_store(g * grp + i, pss[i], h1_t, b1_sb, 0)

            def emit_hidden(c, lname, h_in, h_out, w_sb, b_sb, boff):
                for m in range(KT):
                    ps = pp.tile([P, NB], F32, tag="ps", name=f"ps{lname}_{c}_{m}")
                    for q in range(QT):
                        nc.tensor.matmul(
                            ps[:],
                            w_sb[:, 2 * q : 2 * q + 2, m * P : (m + 1) * P],
                            h_in[:, 2 * q : 2 * q + 2, :],
                            start=(q == 0),
                            stop=(q == QT - 1),
                            perf_mode=DR,
                        )
                    relu_store(m, ps, h_out, b_sb, boff)

            # ---- L1 software-pipelined two chunks ahead ----------------
            emit_l1(0)
            emit_l1(1)

            # ---- constraint bounds from x (independent of the MLP).
            # Emitted here so the DVE-queue work lands after chunk 0/1's h1
            # ReLUs but well before the first raw conversion needs `ub`.
            def col(c):
                return xc_sb[:, c * JT : (c + 1) * JT]

            cem, slag, fly, wat, ager = col(0), col(1), col(2), col(3), col(7)

            def ctile(name):
                return cp.tile([P, JT], F32, tag=name, name=name)

            def mtile(name):
                return cp.tile([P, JT], mybir.dt.uint8, tag=name, name=name)

            vec = nc.vector

            age = ctile("age")
            vec.tensor_single_scalar(age[:], ager, 1.0, ALU.max)
            cmask = mtile("cmask")
            vec.tensor_single_scalar(cmask[:], cem, 0.0, ALU.is_gt)
            wmask = mtile("wmask")
            vec.tensor_single_scalar(wmask[:], wat, 0.0, ALU.is_gt)
            vmask = mtile("vmask")
            vec.tensor_tensor(vmask[:], cmask[:], wmask[:], ALU.bitwise_and)
            ones = ctile("ones")
            vec.memset(ones[:], 1.0)
            cems = ctile("cems")
            vec.select(cems[:], cmask[:], cem, ones[:])
            rcem = ctile("rcem")
            vec.reciprocal(rcem[:], cems[:])
            wc = ctile("wc")
            vec.tensor_tensor(wc[:], wat, rcem[:], ALU.mult)
            scm = ctile("scm")
            vec.tensor_tensor(scm[:], slag, fly, ALU.add)
            binder = ctile("binder")
            vec.tensor_tensor(binder[:], cem, scm[:], ALU.add)
            den1 = ctile("den1")
            vec.tensor_single_scalar(den1[:], binder[:], 0.1, ALU.max)
            rden1 = ctile("rden1")
            vec.reciprocal(rden1[:], den1[:])
            r1s = ctile("r1s")
            vec.tensor_tensor(r1s[:], scm[:], rden1[:], ALU.mult)
            amax = ctile("amax")
            vec.tensor_scalar(amax[:], r1s[:], -0.15, 0.95, ALU.mult, ALU.add)
            hyd = ctile("hyd")
            vec.tensor_single_scalar(hyd[:], wc[:], 1.0, ALU.add)
            rhyd = ctile("rhyd")
            vec.reciprocal(rhyd[:], hyd[:])
            ea = ctile("ea")
            vec.tensor_tensor(ea[:], rhyd[:], age[:], ALU.mult)
            ex = ctile("ex")
            nc.scalar.activation(ex[:], ea[:], AF.Exp, scale=-0.01)
            omex = ctile("omex")
            vec.tensor_scalar(omex[:], ex[:], -1.0, 1.0, ALU.mult, ALU.add)
            alpha = ctile("alpha")
            vec.tensor_tensor(alpha[:], amax[:], omex[:], ALU.mult)
            bmask = mtile("bmask")
            vec.tensor_single_scalar(bmask[:], binder[:], 0.0, ALU.is_gt)
            bsafe = ctile("bsafe")
            vec.select(bsafe[:], bmask[:], binder[:], ones[:])
            rbs = ctile("rbs")
            vec.reciprocal(rbs[:], bsafe[:])
            cf = ctile("cf")
            vec.tensor_tensor(cf[:], cem, rbs[:], ALU.mult)
            acf = ctile("acf")
            vec.tensor_tensor(acf[:], alpha[:], cf[:], ALU.mult)
            wcmask = mtile("wcmask")
            vec.tensor_single_scalar(wcmask[:], wc[:], 0.0, ALU.is_gt)
            wcsafe = ctile("wcsafe")
            vec.select(wcsafe[:], wcmask[:], wc[:], ones[:])
            rwcs = ctile("rwcs")
            vec.reciprocal(rwcs[:], wcsafe[:])
            gel = ctile("gel")
            vec.tensor_tensor(gel[:], acf[:], rwcs[:], ALU.mult)
            g = ctile("g")
            vec.tensor_scalar(g[:], gel[:], 0.01, 10.0, ALU.max, ALU.min)
            g2 = ctile("g2")
            vec.tensor_tensor(g2[:], g[:], g[:], ALU.mult)
            g3 = ctile("g3")
            vec.tensor_tensor(g3[:], g2[:], g[:], ALU.mult)
            phys = ctile("phys")
            vec.tensor_scalar(phys[:], g3[:], 50.0, 5.0, ALU.mult, ALU.max)
            physl = ctile("physl")
            vec.tensor_single_scalar(physl[:], phys[:], 120.0, ALU.min)
            tot1 = ctile("tot1")
            vec.tensor_tensor(tot1[:], cem, wat, ALU.add)
            total = ctile("total")
            vec.tensor_tensor(total[:], tot1[:], scm[:], ALU.add)
            dtot = ctile("dtot")
            vec.tensor_single_scalar(dtot[:], total[:], 1e-6, ALU.max)
            rtot = ctile("rtot")
            vec.reciprocal(rtot[:], dtot[:])
            cfac = ctile("cfac")
            vec.tensor_tensor(cfac[:], cem, rtot[:], ALU.mult)
            cons = ctile("cons")
            vec.tensor_single_scalar(cons[:], cfac[:], 120.0, ALU.mult)
            ub = ctile("ub")
            vec.tensor_tensor(ub[:], physl[:], cons[:], ALU.min)
            amask = mtile("amask")
            vec.tensor_tensor(amask[:], vmask[:], bmask[:], ALU.bitwise_and)

            # ---- raw [1, w] -> batch-major clamp -> store --------------
            raw_bm = cp.tile([P, JT], F32, tag="raw_bm")
            rawb = ctile("rawb")
            lo5 = ctile("lo5")
            constr = ctile("constr")
            outsb = cp.tile([P, JT], F32, tag="outsb")
            nj = NB // P  # batch-major columns produced per chunk

            def raw_to_out(c, ps_part, rows, part_id, eng=None):
                # batch-major mapping: sample n of chunk c lives at
                # (partition n//4, column c*4 + n%4).  `rows` selects the
                # partition range this piece covers (full or half chunk).
                eng = eng or nc.sync
                w = (rows.stop - rows.start) * nj
                rawt = rp.tile([1, w], F32, tag="rawt", name=f"rawt{c}_{part_id}")
                vec.tensor_copy(rawt[:], ps_part)
                sl = slice(c * nj, (c + 1) * nj)
                # single strided SBUF->SBUF DMA does the [1, w] -> [p, 4]
                # batch-major transpose in one hop.
                eng.dma_start(
                    raw_bm[rows, sl],
                    rawt[0:1, :].rearrange("o (p j) -> o p j", j=nj),
                )
                vec.tensor_scalar(
                    rawb[rows, sl], raw_bm[rows, sl], 1.0 / RAW_DIV,
                    b4_sb[rows, 0:1], ALU.mult, ALU.add,
                )
                vec.tensor_single_scalar(lo5[rows, sl], rawb[rows, sl], 5.0, ALU.max)
                vec.tensor_tensor(
                    constr[rows, sl], lo5[rows, sl], ub[rows, sl], ALU.min
                )
                vec.select(
                    outsb[rows, sl], amask[rows, sl], constr[rows, sl],
                    rawb[rows, sl],
                )
                nc.gpsimd.dma_start(out_d[rows, sl], outsb[rows, sl])

            # ---- main chunk loop ---------------------------------------
            for c in range(NCH):
                h1_t = h_tiles[("h1", c)]
                h2_t = hp.tile([P, KT, NB], F8, tag="h2", name=f"h2_{c}", bufs=3)
                emit_hidden(c, "2", h1_t, h2_t, w2_sb, b23_sb, 0)
                if c + 2 < NCH:
                    # L1 lookahead fills the PE while L2's last ReLUs land.
                    emit_l1(c + 2)
                h3_t = hp.tile([P, KT, NB], F8, tag="h3", name=f"h3_{c}", bufs=3)
                emit_hidden(c, "3", h2_t, h3_t, w3_sb, b23_sb, KT)

                if c < NCH - 1:
                    ps4 = pp4.tile([2, NB], F32, tag="ps4", name=f"ps4_{c}")
                    for q in range(QT):
                        nc.tensor.matmul(
                            ps4[:],
                            w4_sb[:, 2 * q : 2 * q + 2, 0:2],
                            h3_t[:, 2 * q : 2 * q + 2, :],
                            start=(q == 0),
                            stop=(q == QT - 1),
                            perf_mode=DR,
                        )
                    raw_to_out(c, ps4[0:1, :], slice(0, P), "a")
                else:
                    # last chunk: split L4 into halves so the first half's
                    # raw conversion overlaps the second half's matmuls.
                    HB = NB // 2
                    ps4a = pp4.tile([2, HB], F32, tag="ps4", name="ps4_la")
                    ps4b = pp.tile([2, HB], F32, tag="ps", name="ps4_lb")
                    for q in range(QT):
                        nc.tensor.matmul(
                            ps4a[:],
                            w4_sb[:, 2 * q : 2 * q + 2, 0:2],
                            h3_t[:, 2 * q : 2 * q + 2, :HB],
                            start=(q == 0),
                            stop=(q == QT - 1),
                            perf_mode=DR,
                        )
                    raw_to_out(c, ps4a[0:1, :], slice(0, P // 2), "a")
                    for q in range(QT):
                        nc.tensor.matmul(
                            ps4b[:],
                            w4_sb[:, 2 * q : 2 * q + 2, 0:2],
                            h3_t[:, 2 * q : 2 * q + 2, HB:],
                            start=(q == 0),
                            stop=(q == QT - 1),
                            perf_mode=DR,
                        )
                    raw_to_out(c, ps4b[0:1, :], slice(P // 2, P), "b", eng=nc.gpsimd)

    nc.compile()
    return nc


def _get_nc():
    if "nc" not in _CACHE:
        _CACHE["nc"] = _build_nc()
    return _CACHE["nc"]


def _prep_in_maps(x, W1, b1, W2, b2, W3, b3, W4, b4):
    f = np.float32
    f8 = mybir.dt.np(F8)
    x = np.ascontiguousarray(np.asarray(x, f))
    W1 = np.asarray(W1, f)
    W2 = np.asarray(W2, f)
    W3 = np.asarray(W3, f)
    W4 = np.asarray(W4, f)
    # [p, k, m] = W[k*128+p, m], scaled + quantized to fp8
    w2p = np.ascontiguousarray(
        (S2 * W2).reshape(KT, P, H).transpose(1, 0, 2).astype(f8)
    )
    w3p = np.ascontiguousarray(
        (S3 * W3).reshape(KT, P, H).transpose(1, 0, 2).astype(f8)
    )
    w4p = np.zeros((P, KT, 16), f8)
    w4c = (S4 * W4).reshape(KT, P).T.astype(f8)
    w4p[:, :, 0] = w4c
    w4p[:, :, 1] = w4c
    b1p = (S1 * np.asarray(b1, f)).reshape(KT, P).T
    b2p = (S1 * S2 * np.asarray(b2, f)).reshape(KT, P).T
    b3p = (S1 * S2 * S3 * np.asarray(b3, f)).reshape(KT, P).T
    b4p = np.full((P, 1), np.asarray(b4, f).reshape(-1)[0], f)

    in_maps = []
    for c in range(N_CORES):
        sl = x[c * BC : (c + 1) * BC]  # [4096, 8]
        # wx = [w1r | xTr]: W1/x^T replicated into partition rows
        # {0-7, 32-39, 64-71, 96-103} for the packed (tile_position) L1.
        wx_c = np.zeros((P, H + BC), f)
        wxv = wx_c.reshape(4, 32, H + BC)
        wxv[:, :D_IN, :H] = S1 * W1
        wxv[:, :D_IN, H:] = sl.T
        # xc[p, f*JT + c*4 + j] = sl[c*512 + p*4 + j, f]
        xc_c = (
            sl.reshape(NCH, P, NB // P, D_IN)
            .transpose(1, 3, 0, 2)
            .reshape(P, D_IN * JT)
        )
        consts_c = np.ascontiguousarray(
            np.concatenate([xc_c, b1p, b2p, b3p, b4p], axis=1)
        )
        in_maps.append(
            {
                "wx": np.ascontiguousarray(wx_c),
                "consts": consts_c,
                "w2": w2p,
                "w3": w3p,
                "w4": w4p,
            }
        )
    return in_maps


def kernel(x, W1, b1, W2, b2, W3, b3, W4, b4, **run_kwargs):
    nc = _get_nc()
    in_maps = _prep_in_maps(x, W1, b1, W2, b2, W3, b3, W4, b4)
    res = run_bass_kernel_spmd(nc, in_maps, core_ids=list(range(N_CORES)), **run_kwargs)
    out = np.empty((B, 1), np.float32)
    for c in range(N_CORES):
        out[c * BC : (c + 1) * BC, 0] = (
            res.results[c]["out_bm"]
            .reshape(P, NCH, NB // P)
            .transpose(1, 0, 2)
            .reshape(BC)
        )
    if run_kwargs:
        kernel.last_results = res
    return out
